# revision 5
# baseline (speedup 1.0000x reference)
"""Binary-weight dense layer on 8 trn2 NeuronCores.

Computes out[b,s,f] = scale * sum_i x[b,s,i] * (kernel[i,f] ? +1 : -1)
for x [4, 4096, 1024] f32, kernel [1024, 1024] bool, scale scalar f32.

Strategy: data-parallel over the 16384 rows (2048 rows/core), pure bf16
matmul with scale folded into the +-1 weights (exact in bf16 for
power-of-two scales).  Host-side prep packs per-core x into
[128, 8k, 2048m] and w into [128, 8k, 1024n] so every DMA line is
contiguous per partition and tiles land directly in matmul layout.

Schedule (from baseline trace analysis):
- 3 DMA rings: sync carries w + output evictions, scalar carries the
  x bulk (rows 512-2047), vector carries the head (w0 half + x rows
  0-511) so the first real matmul's operands land ~1us after the
  Tile start barrier.
- 2 dummy matmuls on a memset buffer bridge the gap until real data
  arrives; the real matmul stream itself then warms the HAM clock
  gate (first ~3.4us run at 1.2 GHz regardless -- dedicating dummy
  matmuls to the whole warmup window just wastes the half-rate work).
- Phase 1 k-major over m-tiles 0-3 consumes chunks in arrival order;
  phase 2 m-major for m-tiles 4-15 (inputs resident by then).
- PSUM f32 is converted to bf16 by the DVE eviction copy, halving
  output DMA bytes; the host upcasts to f32.
"""

import numpy as np
import ml_dtypes

import concourse.bacc as bacc
import concourse.mybir as mybir
import concourse.tile as tile
from concourse.bass_utils import run_bass_kernel_spmd

N_CORES = 8
B, S, K, N = 4, 4096, 1024, 1024
ROWS = B * S                     # 16384
ROWS_PER_CORE = ROWS // N_CORES  # 2048
P = 128                          # partitions
KT = K // P                      # 8 contraction subtiles
MT = ROWS_PER_CORE // P          # 16 row tiles per core
NHALF = 512                      # one PSUM bank of f32
G0 = 4                           # phase-1 m-tiles (PSUM holds exactly 4)
GROWS = G0 * P                   # 512 rows covered by phase 1

_module_cache = {}


def build_module():
    nc = bacc.Bacc(None)
    # Host-packed layouts: per partition p the innermost dims are
    # contiguous in DRAM, so DMA lines are (slice length) wide.
    xt = nc.dram_tensor("xt", [P, KT, ROWS_PER_CORE], mybir.dt.bfloat16,
                        kind="ExternalInput")
    w = nc.dram_tensor("w", [P, KT, N], mybir.dt.bfloat16,
                       kind="ExternalInput")
    out = nc.dram_tensor("out", [ROWS_PER_CORE, N], mybir.dt.bfloat16,
                         kind="ExternalOutput")

    with tile.TileContext(nc) as tc:
        with (
            tc.tile_pool(name="persist", bufs=1) as persist,
            tc.tile_pool(name="psum", bufs=1, space="PSUM") as ps_pool,
            tc.tile_pool(name="outp", bufs=3) as out_pool,
        ):
            # Bridge matmuls: keep the PE busy from block entry until the
            # first real operands land (~1us).  Buffer is memset on GpSimd
            # (reading uninitialized SBUF faults the device).
            wu = persist.tile([P, 384], mybir.dt.bfloat16, tag="wu")
            nc.gpsimd.memset(wu, 0)

            X = persist.tile([P, KT, ROWS_PER_CORE], mybir.dt.bfloat16,
                             tag="x", name="x")
            W = persist.tile([P, KT, N], mybir.dt.bfloat16, tag="w", name="w")

            # --- DMA schedule.  Per-ring FIFO order == need order; each
            # queue runs ~150-190 GB/s with ~3 instructions in flight and
            # ~0.6us completion-sem latency, so the first-needed item must
            # be first in its queue. ---
            # sync ring A: w k=0 halves (gate the first real matmuls),
            # then w per k, then (emitted by evict()) all output stores.
            nc.sync.dma_start(out=W[:, 0:1, 0:NHALF],
                              in_=w[:, 0:1, 0:NHALF])
            nc.sync.dma_start(out=W[:, 0:1, NHALF:N],
                              in_=w[:, 0:1, NHALF:N])
            for k in range(1, KT):
                nc.sync.dma_start(out=W[:, k:k + 1, :], in_=w[:, k:k + 1, :])
            # scalar ring B: x only.  k=0 rows split so the m0 matmul can
            # start ASAP; then g0 per k, rows 512-1023 per k (first
            # phase-2 tiles), then rows 1024-2047 in k pairs (2KB lines).
            nc.scalar.dma_start(out=X[:, 0:1, 0:P], in_=xt[:, 0:1, 0:P])
            nc.scalar.dma_start(out=X[:, 0:1, P:GROWS],
                                in_=xt[:, 0:1, P:GROWS])
            for k in range(1, KT):
                nc.scalar.dma_start(out=X[:, k:k + 1, 0:GROWS],
                                    in_=xt[:, k:k + 1, 0:GROWS])
            for k in range(KT):
                nc.scalar.dma_start(out=X[:, k:k + 1, GROWS:2 * GROWS],
                                    in_=xt[:, k:k + 1, GROWS:2 * GROWS])
            for k in range(0, KT, 2):
                nc.scalar.dma_start(out=X[:, k:k + 2, 2 * GROWS:ROWS_PER_CORE],
                                    in_=xt[:, k:k + 2, 2 * GROWS:ROWS_PER_CORE])

            # --- PE stream ---
            ps_tiles = {}
            for m in range(G0):
                ps_tiles[m] = ps_pool.tile([P, N], mybir.dt.float32,
                                           tag=f"ps{m}", name=f"ps{m}")
            # Bridge matmuls: keep the PE continuously busy (warming the
            # HAM gate) from block entry (~7.2us) until the first real
            # operands land (~10us); a PE idle gap here resets the HAM
            # warmup progress and the real stream then runs at 1.2 GHz.
            for _ in range(11):
                nc.tensor.matmul(ps_tiles[0][:, 0:256], wu[:, 0:P],
                                 wu[:, P:384], start=True, stop=True)

            def mm(m, k):
                lhsT = X[:, k, m * P:(m + 1) * P]
                ps = ps_tiles[m % G0]
                nc.tensor.matmul(ps[:, 0:NHALF], lhsT, W[:, k, 0:NHALF],
                                 start=(k == 0), stop=(k == KT - 1))
                nc.tensor.matmul(ps[:, NHALF:N], lhsT, W[:, k, NHALF:N],
                                 start=(k == 0), stop=(k == KT - 1))

            def evict(m):
                ot = out_pool.tile([P, N], mybir.dt.bfloat16, tag="ot")
                if m == MT - 1:
                    # last tile: halves on both rings so the first half's
                    # store overlaps the second half's copy
                    nc.vector.tensor_copy(ot[:, 0:NHALF],
                                          ps_tiles[m % G0][:, 0:NHALF])
                    nc.sync.dma_start(out=out[m * P:(m + 1) * P, 0:NHALF],
                                      in_=ot[:, 0:NHALF])
                    nc.vector.tensor_copy(ot[:, NHALF:N],
                                          ps_tiles[m % G0][:, NHALF:N])
                    nc.scalar.dma_start(out=out[m * P:(m + 1) * P, NHALF:N],
                                        in_=ot[:, NHALF:N])
                else:
                    nc.vector.tensor_copy(ot, ps_tiles[m % G0])
                    nc.sync.dma_start(out=out[m * P:(m + 1) * P, :], in_=ot)

            # Phase 1: first G0 m-tiles k-major, consuming chunks as they
            # arrive from DMA.
            for k in range(KT):
                for m in range(G0):
                    mm(m, k)
            for m in range(G0):
                evict(m)

            # Phase 2: remaining m-tiles m-major (inputs resident),
            # copy-out pipelined with the next tile's matmuls.  The last
            # tile runs its two n-halves back to back (all k of half a,
            # then all k of half b) so half a's eviction overlaps half b's
            # matmuls and only half b's eviction trails the PE stream.
            for m in range(G0, MT):
                ps_tiles[m % G0] = ps_pool.tile([P, N], mybir.dt.float32,
                                                tag=f"ps{m % G0}",
                                                name=f"ps{m}")
                if m == MT - 1:
                    ps = ps_tiles[m % G0]
                    ot = out_pool.tile([P, N], mybir.dt.bfloat16, tag="ot")
                    for h in range(2):
                        lo, hi = h * NHALF, (h + 1) * NHALF
                        for k in range(KT):
                            nc.tensor.matmul(ps[:, lo:hi],
                                             X[:, k, m * P:(m + 1) * P],
                                             W[:, k, lo:hi],
                                             start=(k == 0), stop=(k == KT - 1))
                        nc.vector.tensor_copy(ot[:, lo:hi], ps[:, lo:hi])
                        ring = nc.sync if h == 0 else nc.scalar
                        ring.dma_start(out=out[m * P:(m + 1) * P, lo:hi],
                                       in_=ot[:, lo:hi])
                else:
                    for k in range(KT):
                        mm(m, k)
                    evict(m)
    nc.finalize()
    return nc


def get_module():
    if "nc" not in _module_cache:
        _module_cache["nc"] = build_module()
    return _module_cache["nc"]


def _prepare_in_maps(x, kernel, scale):
    bf16 = ml_dtypes.bfloat16
    x2d = np.asarray(x, dtype=np.float32).reshape(ROWS, K)
    scale = np.float32(scale)
    w_signed = np.where(np.asarray(kernel, dtype=bool), scale, -scale)
    # w[p, k, n] = w_signed[k*128 + p, n]
    w_packed = np.ascontiguousarray(
        w_signed.reshape(KT, P, N).transpose(1, 0, 2).astype(bf16))
    in_maps = []
    for c in range(N_CORES):
        shard = x2d[c * ROWS_PER_CORE:(c + 1) * ROWS_PER_CORE]
        # xt[p, k, m] = shard[m, k*128 + p]
        xt_c = np.ascontiguousarray(
            shard.T.reshape(KT, P, ROWS_PER_CORE).transpose(1, 0, 2)
            .astype(bf16))
        in_maps.append({"xt": xt_c, "w": w_packed})
    return in_maps


def kernel(x, kernel, scale):
    nc = get_module()
    in_maps = _prepare_in_maps(x, kernel, scale)
    res = run_bass_kernel_spmd(nc, in_maps, core_ids=list(range(N_CORES)))
    out = np.concatenate([r["out"] for r in res.results], axis=0)
    return out.astype(np.float32).reshape(B, S, N)
